# revision 18
# baseline (speedup 1.0000x reference)
"""Causal multi-head self-attention (RoPE) Trainium2 Bass kernel. v2

Sharding: 8 cores = 2 batches x 4 head-groups (4 heads each).
Per core: QKV projections (bf16 in, f32 psum) from host-pretransposed x,
RoPE with host-precomputed sin/cos tables, causal flash-style attention
(S^T orientation, ones-row softmax denominator), O-projection partial,
then per-q-tile ReduceScatter over the 4 cores of each batch.
Projection (A) chunks are woven into the attention (B) q-tile loop to
keep the PE continuously busy (sustained full pstate).

PSUM budget (8 banks): trps 2 + qkv 3 + st 2 + ao 1.
"""

import sys, math

sys.path.insert(0, '/opt/trn_rl_repo')
import numpy as np
import ml_dtypes

B, S, D, H, DK = 2, 2048, 1024, 16, 64
HC = 4            # heads per core
E = HC * DK       # 256 local projection width
NSC = S // 128    # 16 s-chunks
NQT = S // 256    # 8 q-tiles of 256
ROPE_THETA = 10000.0
MASK_VAL = -1e30

_compiled = None


def _build():
    import concourse.bass as bass
    import concourse.tile as tile
    from concourse import bacc, mybir
    from concourse.masks import make_identity

    F32 = mybir.dt.float32
    F32R = mybir.dt.float32r
    BF16 = mybir.dt.bfloat16
    AF = mybir.ActivationFunctionType
    ALU = mybir.AluOpType

    nc = bacc.Bacc()

    xt_d = nc.dram_tensor("xt", [NSC, 128, 8, 128], BF16, kind="ExternalInput")
    wq_d = nc.dram_tensor("wq", [128, 8, E], BF16, kind="ExternalInput")
    wk_d = nc.dram_tensor("wk", [128, 8, E], BF16, kind="ExternalInput")
    wv_d = nc.dram_tensor("wv", [128, 8, E], BF16, kind="ExternalInput")
    wo_d = nc.dram_tensor("wo", [128, 2, D], F32R, kind="ExternalInput")
    cost_d = nc.dram_tensor("cost", [128, NSC, 32], F32, kind="ExternalInput")
    sint_d = nc.dram_tensor("sint", [128, NSC, 32], F32, kind="ExternalInput")
    nsint_d = nc.dram_tensor("nsint", [128, NSC, 32], F32, kind="ExternalInput")
    m01_d = nc.dram_tensor("m01", [128, 512], F32, kind="ExternalInput")
    sel_d = nc.dram_tensor("sel", [4, 256], F32R, kind="ExternalInput")
    idt_d = nc.dram_tensor("idt", [128, 128], F32R, kind="ExternalInput")
    y_d = nc.dram_tensor("y", [512, D], BF16, kind="ExternalOutput")
    cc_in = [nc.dram_tensor(f"cc_in{q}", [512, D], BF16) for q in range(NQT // 2)]
    cc_out = [nc.dram_tensor(f"cc_out{q}", [128, D], BF16) for q in range(NQT // 2)]
    groups = [[0, 1, 2, 3], [4, 5, 6, 7]]

    with tile.TileContext(nc) as tc:
        with (
            tc.tile_pool(name="const", bufs=1) as cp,
            tc.tile_pool(name="big", bufs=1) as bp,
            tc.tile_pool(name="xt", bufs=3) as xtp,
            tc.tile_pool(name="rope", bufs=2) as rp,
            tc.tile_pool(name="pt", bufs=3) as ptp,
            tc.tile_pool(name="outs", bufs=2) as osp,
            tc.tile_pool(name="small", bufs=2) as smp,
            tc.tile_pool(name="trps", bufs=2, space="PSUM") as trps,
            tc.tile_pool(name="qkvps", bufs=1, space="PSUM") as qkvps,
            tc.tile_pool(name="stps", bufs=2, space="PSUM") as stps,
            tc.tile_pool(name="aops", bufs=1, space="PSUM") as aops,
        ):
            # ---- constants / weights
            wq_t = cp.tile([128, 8, E], BF16, tag="wq")
            wk_t = cp.tile([128, 8, E], BF16, tag="wk")
            wv_t = cp.tile([128, 8, E], BF16, tag="wv")
            wo_t = cp.tile([128, 2, D], F32R, tag="wo")
            nc.sync.dma_start(wq_t[:], wq_d[:])
            nc.sync.dma_start(wk_t[:], wk_d[:])
            nc.sync.dma_start(wv_t[:], wv_d[:])
            mask01 = cp.tile([128, 512], F32, tag="m01", name="mask01")
            nc.sync.dma_start(mask01[:], m01_d[:])
            sint = cp.tile([128, NSC, 32], F32, tag="sint")
            cost = cp.tile([128, NSC, 32], F32, tag="cost")
            nsint = cp.tile([128, NSC, 32], F32, tag="nsint")
            nc.sync.dma_start(cost[:], cost_d[:])
            nc.sync.dma_start(sint[:], sint_d[:])
            nc.sync.dma_start(nsint[:], nsint_d[:])
            sel = cp.tile([4, 256], F32R, tag="sel")
            nc.sync.dma_start(sel[:], sel_d[:])
            nc.sync.dma_start(wo_t[:], wo_d[:])
            ident_t = cp.tile([128, 128], F32R, tag="ident")
            nc.sync.dma_start(ident_t[:], idt_d[:])
            ident = ident_t[:]

            # ---- persistent activation tensors
            QT = [bp.tile([128, S], F32R, tag=f"qt{i}", name=f"qt{i}") for i in range(2)]
            KT = [bp.tile([128, S], F32R, tag=f"kt{i}", name=f"kt{i}") for i in range(2)]
            VO = bp.tile([128, NSC, HC * 65], F32R, tag="vo")
            AOT = [bp.tile([128, S], F32R, tag=f"aot{i}", name=f"aot{i}") for i in range(2)]
            # ones column of VO (softmax denominator trick)
            ones_c = cp.tile([128, NSC * HC], F32, tag="ones_c")
            nc.vector.memset(ones_c[:], 1.0)
            with nc.allow_low_precision(reason="f32r ones column"):
                nc.vector.tensor_copy(
                    VO[:].rearrange("p c (h z) -> p c h z", z=65)[:, :, :, 64:65],
                    ones_c[:].rearrange("p (c h a) -> p c h a", h=HC, a=1))

            # ---------------- Phase A units ----------------
            qkv_live = {}

            def a_unit1(sc):
                """x^T chunk DMA + QKV matmuls (bf16 in, f32 psum)."""
                xt = xtp.tile([128, 8, 128], BF16, tag="xt")
                nc.sync.dma_start(xt[:], xt_d[sc])
                q_ps = qkvps.tile([128, E], F32, tag="qp")
                k_ps = qkvps.tile([128, E], F32, tag="kp")
                v_ps = qkvps.tile([128, E], F32, tag="vp")
                for dc in range(8):
                    nc.tensor.matmul(q_ps[:], xt[:, dc, :], wq_t[:, dc, :],
                                     start=(dc == 0), stop=(dc == 7))
                    nc.tensor.matmul(k_ps[:], xt[:, dc, :], wk_t[:, dc, :],
                                     start=(dc == 0), stop=(dc == 7))
                    nc.tensor.matmul(v_ps[:], xt[:, dc, :], wv_t[:, dc, :],
                                     start=(dc == 0), stop=(dc == 7))
                qkv_live[sc] = (q_ps, k_ps, v_ps)

            def a_unit2(sc):
                """RoPE + PE transposes into QT/KT, V copy into VO."""
                q_ps, k_ps, v_ps = qkv_live.pop(sc)
                cosb = cost[:, sc, :].rearrange("p (a f) -> p a f", a=1) \
                    .to_broadcast([128, 8, 32])
                sinb = sint[:, sc, :].rearrange("p (a f) -> p a f", a=1) \
                    .to_broadcast([128, 4, 32])
                nsinb = nsint[:, sc, :].rearrange("p (a f) -> p a f", a=1) \
                    .to_broadcast([128, 4, 32])
                for ti, (src_ps, dst) in enumerate(((q_ps, QT), (k_ps, KT))):
                    src = src_ps[:]
                    t_s = rp.tile([128, E], F32R, tag="t")
                    u_s = rp.tile([128, E], F32R, tag="u")
                    with nc.allow_low_precision(reason="f32r rope staging"):
                        nc.vector.tensor_tensor(
                            out=t_s[:].rearrange("p (a f) -> p a f", f=32),
                            in0=src.rearrange("p (a f) -> p a f", f=32),
                            in1=cosb, op=ALU.mult)
                        s4 = src.rearrange("p (h two f) -> p h two f", two=2, f=32)
                        u4 = u_s[:].rearrange("p (h two f) -> p h two f", two=2, f=32)
                        nc.vector.tensor_tensor(out=u4[:, :, 0, :], in0=s4[:, :, 1, :],
                                                in1=nsinb, op=ALU.mult)
                        nc.vector.tensor_tensor(out=u4[:, :, 1, :], in0=s4[:, :, 0, :],
                                                in1=sinb, op=ALU.mult)
                    for half in range(2):
                        tr2 = trps.tile([128, 128], F32R, tag="tr")
                        nc.tensor.transpose(
                            tr2[:], t_s[:, half * 128:(half + 1) * 128], ident)
                        nc.tensor.matmul(
                            tr2[:], u_s[:, half * 128:(half + 1) * 128], ident,
                            is_transpose=True, start=False, stop=True)
                        if ti == 0:
                            nc.scalar.copy(dst[half][:, sc * 128:(sc + 1) * 128],
                                           tr2[:])
                        else:
                            with nc.allow_low_precision(reason="f32r KT copy"):
                                nc.vector.tensor_copy(
                                    dst[half][:, sc * 128:(sc + 1) * 128], tr2[:])
                with nc.allow_low_precision(reason="f32r V copy"):
                    nc.vector.tensor_copy(
                        VO[:].rearrange("p c (h z) -> p c h z", z=65)[:, sc, :, 0:64],
                        v_ps[:].rearrange("p (h f) -> p h f", f=64))

            a_steps = []
            for c in range(NSC):
                a_steps.append((a_unit1, c))
                a_steps.append((a_unit2, c))
            a_pos = [0]

            def pump_a(n):
                for _ in range(n):
                    if a_pos[0] < len(a_steps):
                        fn, c = a_steps[a_pos[0]]
                        fn(c)
                        a_pos[0] += 1

            # ---------------- Phase B ----------------
            def b_head(qt, h, denb):
                """Attention for one head; writes unnormalized out + denom."""
                hc, hb = h // 2, (h % 2) * 64
                kmax = 2 * qt + 1
                ao = aops.tile([128, 256], F32, tag="ao", name=f"ao{qt}_{h}")
                for kc2 in range(0, kmax + 1, 2):
                    st = stps.tile([128, 512], F32, tag="st")
                    for j in range(2):
                        kc = kc2 + j
                        nc.tensor.matmul(
                            st[:, j * 256:(j + 1) * 256],
                            KT[hc][hb:hb + 64, kc * 128:(kc + 1) * 128],
                            QT[hc][hb:hb + 64, qt * 256:(qt + 1) * 256],
                            start=True, stop=True)
                    if kc2 == 2 * qt:
                        nc.vector.tensor_tensor(out=st[:], in0=st[:],
                                                in1=mask01[:], op=ALU.add)
                    pt = ptp.tile([128, 2, 256], F32R, tag="pt")
                    nc.scalar.activation(pt[:], st[:], AF.Exp,
                                         scale=1.0 / math.sqrt(DK))
                    for j in range(2):
                        kc = kc2 + j
                        nc.tensor.matmul(
                            ao[0:65, :],
                            VO[:, kc, h * 65:(h + 1) * 65],
                            pt[:, j, :],
                            start=(kc == 0), stop=(kc == kmax))
                # stage denominator + unnormalized output; frees ao (bufs=1)
                # (all denoms land on partition 0, 4 col segments)
                if h < 2:
                    nc.scalar.copy(denb[0:1, h * 256:(h + 1) * 256], ao[64:65, :])
                    nc.scalar.copy(AOT[hc][hb:hb + 64, qt * 256:(qt + 1) * 256],
                                   ao[0:64, :])
                else:
                    with nc.allow_low_precision(reason="f32r den/aot stage"):
                        nc.vector.tensor_copy(denb[0:1, h * 256:(h + 1) * 256],
                                              ao[64:65, :])
                        nc.vector.tensor_copy(
                            AOT[hc][hb:hb + 64, qt * 256:(qt + 1) * 256],
                            ao[0:64, :])

            def b_epilogue(qt, denb):
                """Batched reciprocal + broadcast + in-place normalize."""
                # reshape [1,1024] -> [4,256] via sbuf-to-sbuf DMA, then recip
                den4 = smp.tile([4, 256], F32, tag="den4")
                nc.sync.dma_start(den4[:], denb[:])
                recb = smp.tile([4, 256], F32R, tag="recb")
                with nc.allow_low_precision(reason="f32r softmax denom"):
                    nc.vector.reciprocal(recb[:], den4[:])
                for hcp in range(2):
                    rep = aops.tile([128, 256], F32, tag="ao", name=f"rep{qt}_{hcp}")
                    nc.tensor.matmul(rep[:], sel[:, hcp * 128:(hcp + 1) * 128],
                                     recb[:], start=True, stop=True)
                    rep_sb = smp.tile([128, 256], F32R, tag="rep_sb")
                    nc.scalar.copy(rep_sb[:], rep[:])
                    with nc.allow_low_precision(reason="f32r attention output"):
                        for a in range(2):
                            sl = AOT[hcp][a * 64:(a + 1) * 64,
                                          qt * 256:(qt + 1) * 256]
                            nc.vector.tensor_tensor(
                                out=sl, in0=sl,
                                in1=rep_sb[a * 64:(a + 1) * 64, :], op=ALU.mult)

            def b_oproj(qt):
                for scl in (2 * qt, 2 * qt + 1):
                    outs = osp.tile([128, D], BF16, tag="outs")
                    for nb in range(2):
                        op = stps.tile([128, 512], F32, tag="st")
                        for cc in range(2):
                            nc.tensor.matmul(
                                op[:],
                                AOT[cc][:, scl * 128:(scl + 1) * 128],
                                wo_t[:, cc, nb * 512:(nb + 1) * 512],
                                start=(cc == 0), stop=(cc == 1))
                        if nb == 0:
                            nc.scalar.copy(outs[:, nb * 512:(nb + 1) * 512], op[:])
                        else:
                            nc.vector.tensor_copy(outs[:, nb * 512:(nb + 1) * 512],
                                                  op[:])
                    nc.sync.dma_start(
                        cc_in[qt // 2][(scl % 4) * 128:(scl % 4) * 128 + 128, :],
                        outs[:])

            def b_rs(qt):
                if qt % 2 == 0:
                    return
                jj = qt // 2
                nc.gpsimd.collective_compute(
                    "ReduceScatter", ALU.add, replica_groups=groups,
                    ins=[cc_in[jj][:]],
                    outs=[cc_out[jj][:]])
                nc.sync.dma_start(y_d[128 * jj:128 * (jj + 1), :], cc_out[jj][:])

            # ---------------- woven schedule ----------------
            pump_a(4)          # chunks 0,1 ready for B(0)
            prev_denb = None
            for qt in range(NQT):
                denb = smp.tile([1, 1024], F32, tag="denb")
                b_head(qt, 0, denb)
                if qt > 0:
                    b_epilogue(qt - 1, prev_denb)
                b_head(qt, 1, denb)
                if qt > 0:
                    b_oproj(qt - 1)
                pump_a(1)
                b_head(qt, 2, denb)
                if qt > 0:
                    b_rs(qt - 1)
                pump_a(1)
                b_head(qt, 3, denb)
                pump_a(2)
                prev_denb = denb
            b_epilogue(NQT - 1, prev_denb)
            b_oproj(NQT - 1)
            b_rs(NQT - 1)

    nc.compile()
    return nc


def _get_compiled():
    global _compiled
    if _compiled is None:
        _compiled = _build()
    return _compiled


def _host_prep(x, Wq, Wk, Wv, Wo, token_positions):
    bf16 = ml_dtypes.bfloat16
    x = np.asarray(x, np.float32)
    Wq = np.asarray(Wq, np.float32)
    Wk = np.asarray(Wk, np.float32)
    Wv = np.asarray(Wv, np.float32)
    Wo = np.asarray(Wo, np.float32)
    pos = np.asarray(token_positions).astype(np.float64)

    # rotate-half permutation within each head: [evens, odds]
    perm = np.concatenate([np.arange(0, DK, 2), np.arange(1, DK, 2)])

    # RoPE tables in [s%128, s//128, freq] layout, f32
    inv_freq = ROPE_THETA ** (-np.arange(0, DK, 2, dtype=np.float64) / DK)
    ang = pos[:, None] * inv_freq[None, :]            # [S, 32]
    cosf = np.cos(ang).astype(np.float32).reshape(NSC, 128, 32).transpose(1, 0, 2)
    sinf = np.sin(ang).astype(np.float32).reshape(NSC, 128, 32).transpose(1, 0, 2)
    cost = np.ascontiguousarray(cosf)
    sint = np.ascontiguousarray(sinf)
    nsint = np.ascontiguousarray(-sinf)

    kl = np.arange(128)[:, None]
    ql = np.arange(256)[None, :]
    m0 = np.where(kl <= ql, 0.0, MASK_VAL).astype(np.float32)
    m1 = np.where(kl + 128 <= ql, 0.0, MASK_VAL).astype(np.float32)
    m01 = np.ascontiguousarray(np.concatenate([m0, m1], axis=1))

    # denominator broadcast selection matrices
    sel = np.zeros((4, 256), np.float32)
    for hcp in range(2):
        for a in range(2):
            sel[2 * hcp + a,
                hcp * 128 + a * 64: hcp * 128 + (a + 1) * 64] = 1.0
    sel = np.ascontiguousarray(sel)

    in_maps = []
    for c in range(8):
        b, g = c // 4, c % 4
        heads = range(HC * g, HC * (g + 1))
        rowsel = np.concatenate([h * DK + perm for h in heads])
        block = slice(E * g, E * (g + 1))
        # x^T tiled: [sc, p, dc, j] = x[b][sc*128+j, dc*128+p]
        xt = x[b].T.reshape(8, 128, NSC, 128).transpose(2, 1, 0, 3)
        wq = Wq[rowsel, :].T.reshape(8, 128, E).transpose(1, 0, 2)
        wk = Wk[rowsel, :].T.reshape(8, 128, E).transpose(1, 0, 2)
        wv = Wv[block, :].T.reshape(8, 128, E).transpose(1, 0, 2)
        wo = Wo[:, block].T.reshape(2, 128, D).transpose(1, 0, 2)
        in_maps.append({
            "xt": np.ascontiguousarray(xt).astype(bf16),
            "wq": np.ascontiguousarray(wq).astype(bf16),
            "wk": np.ascontiguousarray(wk).astype(bf16),
            "wv": np.ascontiguousarray(wv).astype(bf16),
            "wo": np.ascontiguousarray(wo),
            "cost": cost, "sint": sint, "nsint": nsint,
            "m01": m01, "sel": sel,
            "idt": np.eye(128, dtype=np.float32),
        })
    return in_maps


def kernel(x, Wq, Wk, Wv, Wo, token_positions):
    from concourse.bass_utils import run_bass_kernel_spmd

    nc = _get_compiled()
    in_maps = _host_prep(x, Wq, Wk, Wv, Wo, token_positions)
    res = run_bass_kernel_spmd(nc, in_maps, core_ids=list(range(8)))

    out = np.empty((B, S, D), np.float32)
    for b in range(B):
        for r in range(4):
            shard = np.asarray(res.results[4 * b + r]["y"], np.float32)
            for jj in range(4):
                out[b, 512 * jj + 128 * r: 512 * jj + 128 * (r + 1), :] = \
                    shard[128 * jj:128 * (jj + 1), :]
    return out


# revision 19
# speedup vs baseline: 1.0381x; 1.0381x over previous
"""Causal multi-head self-attention (RoPE) Trainium2 Bass kernel. v2

Sharding: 8 cores = 2 batches x 4 head-groups (4 heads each).
Per core: QKV projections (bf16 in, f32 psum) from host-pretransposed x,
RoPE with host-precomputed sin/cos tables, causal flash-style attention
(S^T orientation, ones-row softmax denominator), O-projection partial,
then per-q-tile ReduceScatter over the 4 cores of each batch.
Projection (A) chunks are woven into the attention (B) q-tile loop to
keep the PE continuously busy (sustained full pstate).

PSUM budget (8 banks): trps 2 + qkv 3 + st 2 + ao 1.
"""

import sys, math

sys.path.insert(0, '/opt/trn_rl_repo')
import numpy as np
import ml_dtypes

B, S, D, H, DK = 2, 2048, 1024, 16, 64
HC = 4            # heads per core
E = HC * DK       # 256 local projection width
NSC = S // 128    # 16 s-chunks
NQT = S // 256    # 8 q-tiles of 256
ROPE_THETA = 10000.0
MASK_VAL = -1e30

_compiled = None


def _build():
    import concourse.bass as bass
    import concourse.tile as tile
    from concourse import bacc, mybir
    from concourse.masks import make_identity

    F32 = mybir.dt.float32
    F32R = mybir.dt.float32r
    BF16 = mybir.dt.bfloat16
    AF = mybir.ActivationFunctionType
    ALU = mybir.AluOpType

    nc = bacc.Bacc()

    xt_d = nc.dram_tensor("xt", [NSC, 128, 8, 128], BF16, kind="ExternalInput")
    wq_d = nc.dram_tensor("wq", [128, 8, E], BF16, kind="ExternalInput")
    wk_d = nc.dram_tensor("wk", [128, 8, E], BF16, kind="ExternalInput")
    wv_d = nc.dram_tensor("wv", [128, 8, E], BF16, kind="ExternalInput")
    wo_d = nc.dram_tensor("wo", [128, 2, D], F32R, kind="ExternalInput")
    cost_d = nc.dram_tensor("cost", [128, NSC, 32], F32, kind="ExternalInput")
    sint_d = nc.dram_tensor("sint", [128, NSC, 32], F32, kind="ExternalInput")
    nsint_d = nc.dram_tensor("nsint", [128, NSC, 32], F32, kind="ExternalInput")
    m01_d = nc.dram_tensor("m01", [128, 512], F32, kind="ExternalInput")
    sel_d = nc.dram_tensor("sel", [4, 256], F32R, kind="ExternalInput")
    idt_d = nc.dram_tensor("idt", [128, 128], F32R, kind="ExternalInput")
    y_d = nc.dram_tensor("y", [512, D], BF16, kind="ExternalOutput")
    cc_in = [nc.dram_tensor(f"cc_in{q}", [256, D], BF16) for q in range(NQT)]
    cc_out = [nc.dram_tensor(f"cc_out{q}", [64, D], BF16) for q in range(NQT)]
    groups = [[0, 1, 2, 3], [4, 5, 6, 7]]

    with tile.TileContext(nc) as tc:
        with (
            tc.tile_pool(name="const", bufs=1) as cp,
            tc.tile_pool(name="big", bufs=1) as bp,
            tc.tile_pool(name="xt", bufs=3) as xtp,
            tc.tile_pool(name="rope", bufs=2) as rp,
            tc.tile_pool(name="pt", bufs=3) as ptp,
            tc.tile_pool(name="outs", bufs=2) as osp,
            tc.tile_pool(name="small", bufs=2) as smp,
            tc.tile_pool(name="trps", bufs=2, space="PSUM") as trps,
            tc.tile_pool(name="qkvps", bufs=1, space="PSUM") as qkvps,
            tc.tile_pool(name="stps", bufs=2, space="PSUM") as stps,
            tc.tile_pool(name="aops", bufs=1, space="PSUM") as aops,
        ):
            # ---- constants / weights
            wq_t = cp.tile([128, 8, E], BF16, tag="wq")
            wk_t = cp.tile([128, 8, E], BF16, tag="wk")
            wv_t = cp.tile([128, 8, E], BF16, tag="wv")
            wo_t = cp.tile([128, 2, D], F32R, tag="wo")
            nc.sync.dma_start(wq_t[:], wq_d[:])
            nc.sync.dma_start(wk_t[:], wk_d[:])
            nc.sync.dma_start(wv_t[:], wv_d[:])
            mask01 = cp.tile([128, 512], F32, tag="m01", name="mask01")
            nc.sync.dma_start(mask01[:], m01_d[:])
            sint = cp.tile([128, NSC, 32], F32, tag="sint")
            cost = cp.tile([128, NSC, 32], F32, tag="cost")
            nsint = cp.tile([128, NSC, 32], F32, tag="nsint")
            nc.sync.dma_start(cost[:], cost_d[:])
            nc.sync.dma_start(sint[:], sint_d[:])
            nc.sync.dma_start(nsint[:], nsint_d[:])
            sel = cp.tile([4, 256], F32R, tag="sel")
            nc.sync.dma_start(sel[:], sel_d[:])
            nc.sync.dma_start(wo_t[:], wo_d[:])
            ident_t = cp.tile([128, 128], F32R, tag="ident")
            nc.sync.dma_start(ident_t[:], idt_d[:])
            ident = ident_t[:]

            # ---- persistent activation tensors
            QT = [bp.tile([128, S], F32R, tag=f"qt{i}", name=f"qt{i}") for i in range(2)]
            KT = [bp.tile([128, S], F32R, tag=f"kt{i}", name=f"kt{i}") for i in range(2)]
            VO = bp.tile([128, NSC, HC * 65], F32R, tag="vo")
            AOT = [bp.tile([128, S], F32R, tag=f"aot{i}", name=f"aot{i}") for i in range(2)]
            # ones column of VO (softmax denominator trick)
            ones_c = cp.tile([128, NSC * HC], F32, tag="ones_c")
            nc.vector.memset(ones_c[:], 1.0)
            with nc.allow_low_precision(reason="f32r ones column"):
                nc.vector.tensor_copy(
                    VO[:].rearrange("p c (h z) -> p c h z", z=65)[:, :, :, 64:65],
                    ones_c[:].rearrange("p (c h a) -> p c h a", h=HC, a=1))

            # ---------------- Phase A units ----------------
            qkv_live = {}

            def a_unit1(sc):
                """x^T chunk DMA + QKV matmuls (bf16 in, f32 psum)."""
                xt = xtp.tile([128, 8, 128], BF16, tag="xt")
                nc.sync.dma_start(xt[:], xt_d[sc])
                q_ps = qkvps.tile([128, E], F32, tag="qp")
                k_ps = qkvps.tile([128, E], F32, tag="kp")
                v_ps = qkvps.tile([128, E], F32, tag="vp")
                for dc in range(8):
                    nc.tensor.matmul(q_ps[:], xt[:, dc, :], wq_t[:, dc, :],
                                     start=(dc == 0), stop=(dc == 7))
                    nc.tensor.matmul(k_ps[:], xt[:, dc, :], wk_t[:, dc, :],
                                     start=(dc == 0), stop=(dc == 7))
                    nc.tensor.matmul(v_ps[:], xt[:, dc, :], wv_t[:, dc, :],
                                     start=(dc == 0), stop=(dc == 7))
                qkv_live[sc] = (q_ps, k_ps, v_ps)

            def a_unit2(sc):
                """RoPE + PE transposes into QT/KT, V copy into VO."""
                q_ps, k_ps, v_ps = qkv_live.pop(sc)
                cosb = cost[:, sc, :].rearrange("p (a f) -> p a f", a=1) \
                    .to_broadcast([128, 8, 32])
                sinb = sint[:, sc, :].rearrange("p (a f) -> p a f", a=1) \
                    .to_broadcast([128, 4, 32])
                nsinb = nsint[:, sc, :].rearrange("p (a f) -> p a f", a=1) \
                    .to_broadcast([128, 4, 32])
                for ti, (src_ps, dst) in enumerate(((q_ps, QT), (k_ps, KT))):
                    src = src_ps[:]
                    t_s = rp.tile([128, E], F32R, tag="t")
                    u_s = rp.tile([128, E], F32R, tag="u")
                    with nc.allow_low_precision(reason="f32r rope staging"):
                        nc.vector.tensor_tensor(
                            out=t_s[:].rearrange("p (a f) -> p a f", f=32),
                            in0=src.rearrange("p (a f) -> p a f", f=32),
                            in1=cosb, op=ALU.mult)
                        s4 = src.rearrange("p (h two f) -> p h two f", two=2, f=32)
                        u4 = u_s[:].rearrange("p (h two f) -> p h two f", two=2, f=32)
                        nc.vector.tensor_tensor(out=u4[:, :, 0, :], in0=s4[:, :, 1, :],
                                                in1=nsinb, op=ALU.mult)
                        nc.vector.tensor_tensor(out=u4[:, :, 1, :], in0=s4[:, :, 0, :],
                                                in1=sinb, op=ALU.mult)
                    for half in range(2):
                        tr2 = trps.tile([128, 128], F32R, tag="tr")
                        nc.tensor.transpose(
                            tr2[:], t_s[:, half * 128:(half + 1) * 128], ident)
                        nc.tensor.matmul(
                            tr2[:], u_s[:, half * 128:(half + 1) * 128], ident,
                            is_transpose=True, start=False, stop=True)
                        if ti == 0:
                            nc.scalar.copy(dst[half][:, sc * 128:(sc + 1) * 128],
                                           tr2[:])
                        else:
                            with nc.allow_low_precision(reason="f32r KT copy"):
                                nc.vector.tensor_copy(
                                    dst[half][:, sc * 128:(sc + 1) * 128], tr2[:])
                with nc.allow_low_precision(reason="f32r V copy"):
                    nc.vector.tensor_copy(
                        VO[:].rearrange("p c (h z) -> p c h z", z=65)[:, sc, :, 0:64],
                        v_ps[:].rearrange("p (h f) -> p h f", f=64))

            a_steps = []
            for c in range(NSC):
                a_steps.append((a_unit1, c))
                a_steps.append((a_unit2, c))
            a_pos = [0]

            def pump_a(n):
                for _ in range(n):
                    if a_pos[0] < len(a_steps):
                        fn, c = a_steps[a_pos[0]]
                        fn(c)
                        a_pos[0] += 1

            # ---------------- Phase B ----------------
            def b_head(qt, h, denb):
                """Attention for one head; writes unnormalized out + denom."""
                hc, hb = h // 2, (h % 2) * 64
                kmax = 2 * qt + 1
                ao = aops.tile([128, 256], F32, tag="ao", name=f"ao{qt}_{h}")
                for kc2 in range(0, kmax + 1, 2):
                    st = stps.tile([128, 512], F32, tag="st")
                    for j in range(2):
                        kc = kc2 + j
                        nc.tensor.matmul(
                            st[:, j * 256:(j + 1) * 256],
                            KT[hc][hb:hb + 64, kc * 128:(kc + 1) * 128],
                            QT[hc][hb:hb + 64, qt * 256:(qt + 1) * 256],
                            start=True, stop=True)
                    if kc2 == 2 * qt:
                        nc.vector.tensor_tensor(out=st[:], in0=st[:],
                                                in1=mask01[:], op=ALU.add)
                    pt = ptp.tile([128, 2, 256], F32R, tag="pt")
                    nc.scalar.activation(pt[:], st[:], AF.Exp,
                                         scale=1.0 / math.sqrt(DK))
                    for j in range(2):
                        kc = kc2 + j
                        nc.tensor.matmul(
                            ao[0:65, :],
                            VO[:, kc, h * 65:(h + 1) * 65],
                            pt[:, j, :],
                            start=(kc == 0), stop=(kc == kmax))
                # stage denominator + unnormalized output; frees ao (bufs=1)
                # (all denoms land on partition 0, 4 col segments)
                if h < 2:
                    nc.scalar.copy(denb[0:1, h * 256:(h + 1) * 256], ao[64:65, :])
                    nc.scalar.copy(AOT[hc][hb:hb + 64, qt * 256:(qt + 1) * 256],
                                   ao[0:64, :])
                else:
                    with nc.allow_low_precision(reason="f32r den/aot stage"):
                        nc.vector.tensor_copy(denb[0:1, h * 256:(h + 1) * 256],
                                              ao[64:65, :])
                        nc.vector.tensor_copy(
                            AOT[hc][hb:hb + 64, qt * 256:(qt + 1) * 256],
                            ao[0:64, :])

            def b_epilogue(qt, denb):
                """Batched reciprocal + broadcast + in-place normalize."""
                # reshape [1,1024] -> [4,256] via sbuf-to-sbuf DMA, then recip
                den4 = smp.tile([4, 256], F32, tag="den4")
                nc.sync.dma_start(den4[:], denb[:])
                recb = smp.tile([4, 256], F32R, tag="recb")
                with nc.allow_low_precision(reason="f32r softmax denom"):
                    nc.vector.reciprocal(recb[:], den4[:])
                for hcp in range(2):
                    rep = aops.tile([128, 256], F32, tag="ao", name=f"rep{qt}_{hcp}")
                    nc.tensor.matmul(rep[:], sel[:, hcp * 128:(hcp + 1) * 128],
                                     recb[:], start=True, stop=True)
                    rep_sb = smp.tile([128, 256], F32R, tag="rep_sb")
                    nc.scalar.copy(rep_sb[:], rep[:])
                    with nc.allow_low_precision(reason="f32r attention output"):
                        for a in range(2):
                            sl = AOT[hcp][a * 64:(a + 1) * 64,
                                          qt * 256:(qt + 1) * 256]
                            nc.vector.tensor_tensor(
                                out=sl, in0=sl,
                                in1=rep_sb[a * 64:(a + 1) * 64, :], op=ALU.mult)

            def b_oproj(qt):
                for scl in (2 * qt, 2 * qt + 1):
                    outs = osp.tile([128, D], BF16, tag="outs")
                    for nb in range(2):
                        op = stps.tile([128, 512], F32, tag="st")
                        for cc in range(2):
                            nc.tensor.matmul(
                                op[:],
                                AOT[cc][:, scl * 128:(scl + 1) * 128],
                                wo_t[:, cc, nb * 512:(nb + 1) * 512],
                                start=(cc == 0), stop=(cc == 1))
                        if nb == 0:
                            nc.scalar.copy(outs[:, nb * 512:(nb + 1) * 512], op[:])
                        else:
                            nc.vector.tensor_copy(outs[:, nb * 512:(nb + 1) * 512],
                                                  op[:])
                    nc.sync.dma_start(
                        cc_in[qt][(scl % 2) * 128:(scl % 2) * 128 + 128, :],
                        outs[:])

            def b_rs(qt):
                nc.gpsimd.collective_compute(
                    "ReduceScatter", ALU.add, replica_groups=groups,
                    ins=[cc_in[qt][:]],
                    outs=[cc_out[qt][:]])
                nc.sync.dma_start(y_d[64 * qt:64 * (qt + 1), :], cc_out[qt][:])

            # ---------------- woven schedule ----------------
            pump_a(4)          # chunks 0,1 ready for B(0)
            prev_denb = None
            for qt in range(NQT):
                denb = smp.tile([1, 1024], F32, tag="denb")
                b_head(qt, 0, denb)
                if qt > 0:
                    b_epilogue(qt - 1, prev_denb)
                b_head(qt, 1, denb)
                if qt > 0:
                    b_oproj(qt - 1)
                pump_a(1)
                b_head(qt, 2, denb)
                if qt > 0:
                    b_rs(qt - 1)
                pump_a(1)
                b_head(qt, 3, denb)
                pump_a(2)
                prev_denb = denb
            b_epilogue(NQT - 1, prev_denb)
            b_oproj(NQT - 1)
            b_rs(NQT - 1)

    nc.compile()
    return nc


def _get_compiled():
    global _compiled
    if _compiled is None:
        _compiled = _build()
    return _compiled


def _host_prep(x, Wq, Wk, Wv, Wo, token_positions):
    bf16 = ml_dtypes.bfloat16
    x = np.asarray(x, np.float32)
    Wq = np.asarray(Wq, np.float32)
    Wk = np.asarray(Wk, np.float32)
    Wv = np.asarray(Wv, np.float32)
    Wo = np.asarray(Wo, np.float32)
    pos = np.asarray(token_positions).astype(np.float64)

    # rotate-half permutation within each head: [evens, odds]
    perm = np.concatenate([np.arange(0, DK, 2), np.arange(1, DK, 2)])

    # RoPE tables in [s%128, s//128, freq] layout, f32
    inv_freq = ROPE_THETA ** (-np.arange(0, DK, 2, dtype=np.float64) / DK)
    ang = pos[:, None] * inv_freq[None, :]            # [S, 32]
    cosf = np.cos(ang).astype(np.float32).reshape(NSC, 128, 32).transpose(1, 0, 2)
    sinf = np.sin(ang).astype(np.float32).reshape(NSC, 128, 32).transpose(1, 0, 2)
    cost = np.ascontiguousarray(cosf)
    sint = np.ascontiguousarray(sinf)
    nsint = np.ascontiguousarray(-sinf)

    kl = np.arange(128)[:, None]
    ql = np.arange(256)[None, :]
    m0 = np.where(kl <= ql, 0.0, MASK_VAL).astype(np.float32)
    m1 = np.where(kl + 128 <= ql, 0.0, MASK_VAL).astype(np.float32)
    m01 = np.ascontiguousarray(np.concatenate([m0, m1], axis=1))

    # denominator broadcast selection matrices
    sel = np.zeros((4, 256), np.float32)
    for hcp in range(2):
        for a in range(2):
            sel[2 * hcp + a,
                hcp * 128 + a * 64: hcp * 128 + (a + 1) * 64] = 1.0
    sel = np.ascontiguousarray(sel)

    in_maps = []
    for c in range(8):
        b, g = c // 4, c % 4
        heads = range(HC * g, HC * (g + 1))
        rowsel = np.concatenate([h * DK + perm for h in heads])
        block = slice(E * g, E * (g + 1))
        # x^T tiled: [sc, p, dc, j] = x[b][sc*128+j, dc*128+p]
        xt = x[b].T.reshape(8, 128, NSC, 128).transpose(2, 1, 0, 3)
        wq = Wq[rowsel, :].T.reshape(8, 128, E).transpose(1, 0, 2)
        wk = Wk[rowsel, :].T.reshape(8, 128, E).transpose(1, 0, 2)
        wv = Wv[block, :].T.reshape(8, 128, E).transpose(1, 0, 2)
        wo = Wo[:, block].T.reshape(2, 128, D).transpose(1, 0, 2)
        in_maps.append({
            "xt": np.ascontiguousarray(xt).astype(bf16),
            "wq": np.ascontiguousarray(wq).astype(bf16),
            "wk": np.ascontiguousarray(wk).astype(bf16),
            "wv": np.ascontiguousarray(wv).astype(bf16),
            "wo": np.ascontiguousarray(wo),
            "cost": cost, "sint": sint, "nsint": nsint,
            "m01": m01, "sel": sel,
            "idt": np.eye(128, dtype=np.float32),
        })
    return in_maps


def kernel(x, Wq, Wk, Wv, Wo, token_positions):
    from concourse.bass_utils import run_bass_kernel_spmd

    nc = _get_compiled()
    in_maps = _host_prep(x, Wq, Wk, Wv, Wo, token_positions)
    res = run_bass_kernel_spmd(nc, in_maps, core_ids=list(range(8)))

    out = np.empty((B, S, D), np.float32)
    for b in range(B):
        for r in range(4):
            shard = np.asarray(res.results[4 * b + r]["y"], np.float32)
            for qt in range(NQT):
                out[b, 256 * qt + 64 * r: 256 * qt + 64 * (r + 1), :] = \
                    shard[64 * qt:64 * (qt + 1), :]
    return out


# revision 20
# speedup vs baseline: 1.0912x; 1.0512x over previous
"""Causal multi-head self-attention (RoPE) Trainium2 Bass kernel. v2

Sharding: 8 cores = 2 batches x 4 head-groups (4 heads each).
Per core: QKV projections (bf16 in, f32 psum) from host-pretransposed x,
RoPE with host-precomputed sin/cos tables, causal flash-style attention
(S^T orientation, ones-row softmax denominator), O-projection partial,
then per-q-tile ReduceScatter over the 4 cores of each batch.
Projection (A) chunks are woven into the attention (B) q-tile loop to
keep the PE continuously busy (sustained full pstate).

PSUM budget (8 banks): trps 2 + qkv 3 + st 2 + ao 1.
"""

import sys, math

sys.path.insert(0, '/opt/trn_rl_repo')
import numpy as np
import ml_dtypes

B, S, D, H, DK = 2, 2048, 1024, 16, 64
HC = 4            # heads per core
E = HC * DK       # 256 local projection width
NSC = S // 128    # 16 s-chunks
NQT = S // 256    # 8 q-tiles of 256
ROPE_THETA = 10000.0
MASK_VAL = -1e30

_compiled = None


def _build():
    import concourse.bass as bass
    import concourse.tile as tile
    from concourse import bacc, mybir
    from concourse.masks import make_identity

    F32 = mybir.dt.float32
    F32R = mybir.dt.float32r
    BF16 = mybir.dt.bfloat16
    AF = mybir.ActivationFunctionType
    ALU = mybir.AluOpType

    nc = bacc.Bacc()

    xt_d = nc.dram_tensor("xt", [NSC, 128, 8, 128], BF16, kind="ExternalInput")
    wq_d = nc.dram_tensor("wq", [128, 8, E], BF16, kind="ExternalInput")
    wk_d = nc.dram_tensor("wk", [128, 8, E], BF16, kind="ExternalInput")
    wv_d = nc.dram_tensor("wv", [128, 8, E], BF16, kind="ExternalInput")
    wo_d = nc.dram_tensor("wo", [128, 2, D], F32R, kind="ExternalInput")
    cost_d = nc.dram_tensor("cost", [128, NSC, 32], F32, kind="ExternalInput")
    sint_d = nc.dram_tensor("sint", [128, NSC, 32], F32, kind="ExternalInput")
    nsint_d = nc.dram_tensor("nsint", [128, NSC, 32], F32, kind="ExternalInput")
    m01_d = nc.dram_tensor("m01", [128, 512], F32, kind="ExternalInput")
    sel_d = nc.dram_tensor("sel", [4, 256], F32R, kind="ExternalInput")
    idt_d = nc.dram_tensor("idt", [128, 128], F32R, kind="ExternalInput")
    y_d = nc.dram_tensor("y", [512, D], BF16, kind="ExternalOutput")
    cc_in = [nc.dram_tensor(f"cc_in{q}", [256, D], BF16) for q in range(NQT)]
    cc_out = [nc.dram_tensor(f"cc_out{q}", [64, D], BF16) for q in range(NQT)]
    groups = [[0, 1, 2, 3], [4, 5, 6, 7]]

    with tile.TileContext(nc) as tc:
        with (
            tc.tile_pool(name="const", bufs=1) as cp,
            tc.tile_pool(name="big", bufs=1) as bp,
            tc.tile_pool(name="xt", bufs=3) as xtp,
            tc.tile_pool(name="rope", bufs=2) as rp,
            tc.tile_pool(name="pt", bufs=3) as ptp,
            tc.tile_pool(name="outs", bufs=2) as osp,
            tc.tile_pool(name="small", bufs=2) as smp,
            tc.tile_pool(name="trps", bufs=2, space="PSUM") as trps,
            tc.tile_pool(name="qkvps", bufs=1, space="PSUM") as qkvps,
            tc.tile_pool(name="stps", bufs=2, space="PSUM") as stps,
            tc.tile_pool(name="aops", bufs=1, space="PSUM") as aops,
        ):
            # ---- constants / weights
            wq_t = cp.tile([128, 8, E], BF16, tag="wq")
            wk_t = cp.tile([128, 8, E], BF16, tag="wk")
            wv_t = cp.tile([128, 8, E], BF16, tag="wv")
            wo_t = cp.tile([128, 2, D], F32R, tag="wo")
            nc.sync.dma_start(wq_t[:], wq_d[:])
            nc.sync.dma_start(wk_t[:], wk_d[:])
            nc.sync.dma_start(wv_t[:], wv_d[:])
            ident_t = cp.tile([128, 128], F32R, tag="ident")
            nc.sync.dma_start(ident_t[:], idt_d[:])
            ident = ident_t[:]
            mask01 = cp.tile([128, 512], F32, tag="m01", name="mask01")
            sint = cp.tile([128, NSC, 32], F32, tag="sint")
            cost = cp.tile([128, NSC, 32], F32, tag="cost")
            nsint = cp.tile([128, NSC, 32], F32, tag="nsint")
            sel = cp.tile([4, 256], F32R, tag="sel")

            def late_consts():
                nc.sync.dma_start(cost[:], cost_d[:])
                nc.sync.dma_start(sint[:], sint_d[:])
                nc.sync.dma_start(nsint[:], nsint_d[:])
                nc.sync.dma_start(mask01[:], m01_d[:])
                nc.sync.dma_start(sel[:], sel_d[:])
                nc.sync.dma_start(wo_t[:], wo_d[:])

            # ---- persistent activation tensors
            QT = [bp.tile([128, S], F32R, tag=f"qt{i}", name=f"qt{i}") for i in range(2)]
            KT = [bp.tile([128, S], F32R, tag=f"kt{i}", name=f"kt{i}") for i in range(2)]
            VO = bp.tile([128, NSC, HC * 65], F32R, tag="vo")
            AOT = [bp.tile([128, S], F32R, tag=f"aot{i}", name=f"aot{i}") for i in range(2)]
            # ones column of VO (softmax denominator trick)
            ones_c = cp.tile([128, NSC * HC], F32, tag="ones_c")
            nc.vector.memset(ones_c[:], 1.0)
            with nc.allow_low_precision(reason="f32r ones column"):
                nc.vector.tensor_copy(
                    VO[:].rearrange("p c (h z) -> p c h z", z=65)[:, :, :, 64:65],
                    ones_c[:].rearrange("p (c h a) -> p c h a", h=HC, a=1))

            # ---------------- Phase A units ----------------
            qkv_live = {}

            def a_unit1(sc):
                """x^T chunk DMA + QKV matmuls (bf16 in, f32 psum)."""
                xt = xtp.tile([128, 8, 128], BF16, tag="xt")
                nc.sync.dma_start(xt[:], xt_d[sc])
                q_ps = qkvps.tile([128, E], F32, tag="qp")
                k_ps = qkvps.tile([128, E], F32, tag="kp")
                v_ps = qkvps.tile([128, E], F32, tag="vp")
                for dc in range(8):
                    nc.tensor.matmul(q_ps[:], xt[:, dc, :], wq_t[:, dc, :],
                                     start=(dc == 0), stop=(dc == 7))
                    nc.tensor.matmul(k_ps[:], xt[:, dc, :], wk_t[:, dc, :],
                                     start=(dc == 0), stop=(dc == 7))
                    nc.tensor.matmul(v_ps[:], xt[:, dc, :], wv_t[:, dc, :],
                                     start=(dc == 0), stop=(dc == 7))
                qkv_live[sc] = (q_ps, k_ps, v_ps)

            def a_unit2(sc):
                """RoPE + PE transposes into QT/KT, V copy into VO."""
                q_ps, k_ps, v_ps = qkv_live.pop(sc)
                cosb = cost[:, sc, :].rearrange("p (a f) -> p a f", a=1) \
                    .to_broadcast([128, 8, 32])
                sinb = sint[:, sc, :].rearrange("p (a f) -> p a f", a=1) \
                    .to_broadcast([128, 4, 32])
                nsinb = nsint[:, sc, :].rearrange("p (a f) -> p a f", a=1) \
                    .to_broadcast([128, 4, 32])
                for ti, (src_ps, dst) in enumerate(((q_ps, QT), (k_ps, KT))):
                    src = src_ps[:]
                    t_s = rp.tile([128, E], F32R, tag="t")
                    u_s = rp.tile([128, E], F32R, tag="u")
                    with nc.allow_low_precision(reason="f32r rope staging"):
                        nc.vector.tensor_tensor(
                            out=t_s[:].rearrange("p (a f) -> p a f", f=32),
                            in0=src.rearrange("p (a f) -> p a f", f=32),
                            in1=cosb, op=ALU.mult)
                        s4 = src.rearrange("p (h two f) -> p h two f", two=2, f=32)
                        u4 = u_s[:].rearrange("p (h two f) -> p h two f", two=2, f=32)
                        nc.vector.tensor_tensor(out=u4[:, :, 0, :], in0=s4[:, :, 1, :],
                                                in1=nsinb, op=ALU.mult)
                        nc.vector.tensor_tensor(out=u4[:, :, 1, :], in0=s4[:, :, 0, :],
                                                in1=sinb, op=ALU.mult)
                    for half in range(2):
                        tr2 = trps.tile([128, 128], F32R, tag="tr")
                        nc.tensor.transpose(
                            tr2[:], t_s[:, half * 128:(half + 1) * 128], ident)
                        nc.tensor.matmul(
                            tr2[:], u_s[:, half * 128:(half + 1) * 128], ident,
                            is_transpose=True, start=False, stop=True)
                        if ti == 0:
                            nc.scalar.copy(dst[half][:, sc * 128:(sc + 1) * 128],
                                           tr2[:])
                        else:
                            with nc.allow_low_precision(reason="f32r KT copy"):
                                nc.vector.tensor_copy(
                                    dst[half][:, sc * 128:(sc + 1) * 128], tr2[:])
                with nc.allow_low_precision(reason="f32r V copy"):
                    nc.vector.tensor_copy(
                        VO[:].rearrange("p c (h z) -> p c h z", z=65)[:, sc, :, 0:64],
                        v_ps[:].rearrange("p (h f) -> p h f", f=64))

            a_steps = []
            for c in range(NSC):
                a_steps.append((a_unit1, c))
                a_steps.append((a_unit2, c))
            a_pos = [0]

            def pump_a(n):
                for _ in range(n):
                    if a_pos[0] < len(a_steps):
                        fn, c = a_steps[a_pos[0]]
                        fn(c)
                        a_pos[0] += 1

            # ---------------- Phase B ----------------
            def b_head(qt, h, denb):
                """Attention for one head; writes unnormalized out + denom."""
                hc, hb = h // 2, (h % 2) * 64
                kmax = 2 * qt + 1
                ao = aops.tile([128, 256], F32, tag="ao", name=f"ao{qt}_{h}")
                for kc2 in range(0, kmax + 1, 2):
                    st = stps.tile([128, 512], F32, tag="st")
                    for j in range(2):
                        kc = kc2 + j
                        nc.tensor.matmul(
                            st[:, j * 256:(j + 1) * 256],
                            KT[hc][hb:hb + 64, kc * 128:(kc + 1) * 128],
                            QT[hc][hb:hb + 64, qt * 256:(qt + 1) * 256],
                            start=True, stop=True)
                    if kc2 == 2 * qt:
                        nc.vector.tensor_tensor(out=st[:], in0=st[:],
                                                in1=mask01[:], op=ALU.add)
                    pt = ptp.tile([128, 2, 256], F32R, tag="pt")
                    nc.scalar.activation(pt[:], st[:], AF.Exp,
                                         scale=1.0 / math.sqrt(DK))
                    for j in range(2):
                        kc = kc2 + j
                        nc.tensor.matmul(
                            ao[0:65, :],
                            VO[:, kc, h * 65:(h + 1) * 65],
                            pt[:, j, :],
                            start=(kc == 0), stop=(kc == kmax))
                # stage denominator + unnormalized output; frees ao (bufs=1)
                # (all denoms land on partition 0, 4 col segments)
                if h < 2:
                    nc.scalar.copy(denb[0:1, h * 256:(h + 1) * 256], ao[64:65, :])
                    nc.scalar.copy(AOT[hc][hb:hb + 64, qt * 256:(qt + 1) * 256],
                                   ao[0:64, :])
                else:
                    with nc.allow_low_precision(reason="f32r den/aot stage"):
                        nc.vector.tensor_copy(denb[0:1, h * 256:(h + 1) * 256],
                                              ao[64:65, :])
                        nc.vector.tensor_copy(
                            AOT[hc][hb:hb + 64, qt * 256:(qt + 1) * 256],
                            ao[0:64, :])

            def b_epilogue(qt, denb):
                """Batched reciprocal + broadcast + in-place normalize."""
                # reshape [1,1024] -> [4,256] via sbuf-to-sbuf DMA, then recip
                den4 = smp.tile([4, 256], F32, tag="den4")
                nc.sync.dma_start(den4[:], denb[:])
                recb = smp.tile([4, 256], F32R, tag="recb")
                with nc.allow_low_precision(reason="f32r softmax denom"):
                    nc.vector.reciprocal(recb[:], den4[:])
                for hcp in range(2):
                    rep = aops.tile([128, 256], F32, tag="ao", name=f"rep{qt}_{hcp}")
                    nc.tensor.matmul(rep[:], sel[:, hcp * 128:(hcp + 1) * 128],
                                     recb[:], start=True, stop=True)
                    rep_sb = smp.tile([128, 256], F32R, tag="rep_sb")
                    nc.scalar.copy(rep_sb[:], rep[:])
                    with nc.allow_low_precision(reason="f32r attention output"):
                        for a in range(2):
                            sl = AOT[hcp][a * 64:(a + 1) * 64,
                                          qt * 256:(qt + 1) * 256]
                            nc.vector.tensor_tensor(
                                out=sl, in0=sl,
                                in1=rep_sb[a * 64:(a + 1) * 64, :], op=ALU.mult)

            def b_oproj(qt):
                for scl in (2 * qt, 2 * qt + 1):
                    outs = osp.tile([128, D], BF16, tag="outs")
                    for nb in range(2):
                        op = stps.tile([128, 512], F32, tag="st")
                        for cc in range(2):
                            nc.tensor.matmul(
                                op[:],
                                AOT[cc][:, scl * 128:(scl + 1) * 128],
                                wo_t[:, cc, nb * 512:(nb + 1) * 512],
                                start=(cc == 0), stop=(cc == 1))
                        if nb == 0:
                            nc.scalar.copy(outs[:, nb * 512:(nb + 1) * 512], op[:])
                        else:
                            nc.vector.tensor_copy(outs[:, nb * 512:(nb + 1) * 512],
                                                  op[:])
                    nc.sync.dma_start(
                        cc_in[qt][(scl % 2) * 128:(scl % 2) * 128 + 128, :],
                        outs[:])

            def b_rs(qt):
                nc.gpsimd.collective_compute(
                    "ReduceScatter", ALU.add, replica_groups=groups,
                    ins=[cc_in[qt][:]],
                    outs=[cc_out[qt][:]])
                nc.sync.dma_start(y_d[64 * qt:64 * (qt + 1), :], cc_out[qt][:])

            # ---------------- woven schedule ----------------
            pump_a(1)          # xt(0) DMA + QKV(0) queued first
            late_consts()      # tables/masks/sel/wo behind the hot path
            pump_a(3)          # chunks 0,1 ready for B(0)
            prev_denb = None
            for qt in range(NQT):
                denb = smp.tile([1, 1024], F32, tag="denb")
                b_head(qt, 0, denb)
                if qt > 0:
                    b_epilogue(qt - 1, prev_denb)
                b_head(qt, 1, denb)
                if qt > 0:
                    b_oproj(qt - 1)
                    b_rs(qt - 1)
                pump_a(1)
                b_head(qt, 2, denb)
                pump_a(1)
                b_head(qt, 3, denb)
                pump_a(2)
                prev_denb = denb
            b_epilogue(NQT - 1, prev_denb)
            b_oproj(NQT - 1)
            b_rs(NQT - 1)

    nc.compile()
    return nc


def _get_compiled():
    global _compiled
    if _compiled is None:
        _compiled = _build()
    return _compiled


def _host_prep(x, Wq, Wk, Wv, Wo, token_positions):
    bf16 = ml_dtypes.bfloat16
    x = np.asarray(x, np.float32)
    Wq = np.asarray(Wq, np.float32)
    Wk = np.asarray(Wk, np.float32)
    Wv = np.asarray(Wv, np.float32)
    Wo = np.asarray(Wo, np.float32)
    pos = np.asarray(token_positions).astype(np.float64)

    # rotate-half permutation within each head: [evens, odds]
    perm = np.concatenate([np.arange(0, DK, 2), np.arange(1, DK, 2)])

    # RoPE tables in [s%128, s//128, freq] layout, f32
    inv_freq = ROPE_THETA ** (-np.arange(0, DK, 2, dtype=np.float64) / DK)
    ang = pos[:, None] * inv_freq[None, :]            # [S, 32]
    cosf = np.cos(ang).astype(np.float32).reshape(NSC, 128, 32).transpose(1, 0, 2)
    sinf = np.sin(ang).astype(np.float32).reshape(NSC, 128, 32).transpose(1, 0, 2)
    cost = np.ascontiguousarray(cosf)
    sint = np.ascontiguousarray(sinf)
    nsint = np.ascontiguousarray(-sinf)

    kl = np.arange(128)[:, None]
    ql = np.arange(256)[None, :]
    m0 = np.where(kl <= ql, 0.0, MASK_VAL).astype(np.float32)
    m1 = np.where(kl + 128 <= ql, 0.0, MASK_VAL).astype(np.float32)
    m01 = np.ascontiguousarray(np.concatenate([m0, m1], axis=1))

    # denominator broadcast selection matrices
    sel = np.zeros((4, 256), np.float32)
    for hcp in range(2):
        for a in range(2):
            sel[2 * hcp + a,
                hcp * 128 + a * 64: hcp * 128 + (a + 1) * 64] = 1.0
    sel = np.ascontiguousarray(sel)

    in_maps = []
    for c in range(8):
        b, g = c // 4, c % 4
        heads = range(HC * g, HC * (g + 1))
        rowsel = np.concatenate([h * DK + perm for h in heads])
        block = slice(E * g, E * (g + 1))
        # x^T tiled: [sc, p, dc, j] = x[b][sc*128+j, dc*128+p]
        xt = x[b].T.reshape(8, 128, NSC, 128).transpose(2, 1, 0, 3)
        wq = Wq[rowsel, :].T.reshape(8, 128, E).transpose(1, 0, 2)
        wk = Wk[rowsel, :].T.reshape(8, 128, E).transpose(1, 0, 2)
        wv = Wv[block, :].T.reshape(8, 128, E).transpose(1, 0, 2)
        wo = Wo[:, block].T.reshape(2, 128, D).transpose(1, 0, 2)
        in_maps.append({
            "xt": np.ascontiguousarray(xt).astype(bf16),
            "wq": np.ascontiguousarray(wq).astype(bf16),
            "wk": np.ascontiguousarray(wk).astype(bf16),
            "wv": np.ascontiguousarray(wv).astype(bf16),
            "wo": np.ascontiguousarray(wo),
            "cost": cost, "sint": sint, "nsint": nsint,
            "m01": m01, "sel": sel,
            "idt": np.eye(128, dtype=np.float32),
        })
    return in_maps


def kernel(x, Wq, Wk, Wv, Wo, token_positions):
    from concourse.bass_utils import run_bass_kernel_spmd

    nc = _get_compiled()
    in_maps = _host_prep(x, Wq, Wk, Wv, Wo, token_positions)
    res = run_bass_kernel_spmd(nc, in_maps, core_ids=list(range(8)))

    out = np.empty((B, S, D), np.float32)
    for b in range(B):
        for r in range(4):
            shard = np.asarray(res.results[4 * b + r]["y"], np.float32)
            for qt in range(NQT):
                out[b, 256 * qt + 64 * r: 256 * qt + 64 * (r + 1), :] = \
                    shard[64 * qt:64 * (qt + 1), :]
    return out
